# revision 13
# baseline (speedup 1.0000x reference)
"""KKT-loss Bass kernel for Trainium2, data-parallel over the batch on 8 cores.

Per core (4 samples of A [2048, 2048] fp32):
  - SWDGE DMA loads A in 1 MiB row-chunks [128, 2048], casting fp32 -> bf16
    inline (HBM reads stay fp32, so the kernel is honestly memory-bound on
    the fp32 tensor; SBUF receives bf16).
  - TensorE computes A^T @ lam with the bf16 chunk as the *moving* operand
    (contraction over the partition dim m), accumulating [1, 512] PSUM
    tiles across the 16 row-chunks; a final K=1 matmul adds c_pad into the
    accumulator, then ScalarE squares + reduces PSUM into per-(sample,
    n-chunk) scalars (the stat-loss partial sums).
  - VectorE computes A @ x via fused scalar_tensor_tensor (multiply by the
    row-broadcast x, reduce along the free dim) -> one [128, 1] column of
    Ax per chunk.
  - Small [128, 64]-shaped ops produce per-partition partial sums for the
    primal/dual/comp losses; the host combines all partials.

Host-side prep is layout-only sharding: batch split, x broadcast to 128
partitions (bf16), lambda/b reshaped to [128, 64] column layout.
"""

import os

import numpy as np

from contextlib import ExitStack

import concourse.bacc as bacc
import concourse.tile as tile
import concourse.mybir as mybir
from concourse.bass_utils import run_bass_kernel_spmd

dt = mybir.dt
AOT = mybir.AluOpType
AFT = mybir.ActivationFunctionType

B, M, N = 32, 2048, 2048
N_CORES = 8
S = B // N_CORES            # 4 samples per core
MC = M // 128               # 16 row-chunks per sample
G = S * MC                  # 64 columns in the [128, G] layouts
FREE = 512
NCH = N // FREE             # 4 psum tiles per sample
W_PRIMAL, W_DUAL, W_STAT, W_COMP = 0.1, 0.1, 0.6, 0.2

TRACE = os.environ.get("KKT_TRACE", "0") == "1"

_CACHE = {}


def _build():
    f32 = dt.float32
    bf16 = dt.bfloat16

    nc = bacc.Bacc("TRN2", target_bir_lowering=False, debug=False)
    a_d = nc.dram_tensor("a_in", [S, M, N], f32, kind="ExternalInput").ap()
    x_d = nc.dram_tensor("x_in", [1, S * N], f32, kind="ExternalInput").ap()
    lamc_d = nc.dram_tensor("lamcols_in", [128, G], f32, kind="ExternalInput").ap()
    bc_d = nc.dram_tensor("bcols_in", [128, G], f32, kind="ExternalInput").ap()
    crow_d = nc.dram_tensor("crow_in", [1, S * N], f32, kind="ExternalInput").ap()
    part_d = nc.dram_tensor("part_out", [128, 2 * S + 1], f32,
                            kind="ExternalOutput").ap()
    stat_d = nc.dram_tensor("stat_out", [S * NCH], f32, kind="ExternalOutput").ap()

    with tile.TileContext(nc) as tc, ExitStack() as ctx:
        acast = ctx.enter_context(tc.tile_pool(name="ac", bufs=8))
        scr = ctx.enter_context(tc.tile_pool(name="scr", bufs=2))
        small = ctx.enter_context(tc.tile_pool(name="small", bufs=1))
        pps = ctx.enter_context(tc.tile_pool(name="ps", bufs=2, space="PSUM"))

        lam_cols = small.tile([128, G], f32)
        nc.sync.dma_start(lam_cols[:], lamc_d)
        b_cols = small.tile([128, G], f32)
        nc.sync.dma_start(b_cols[:], bc_d)
        lam_cols_c = small.tile([128, G], bf16)
        nc.vector.tensor_copy(out=lam_cols_c[:], in_=lam_cols[:])

        c_row = small.tile([1, S * N], f32)
        nc.sync.dma_start(c_row[:], crow_d)
        c_row_c = small.tile([1, S * N], bf16)
        nc.scalar.copy(out=c_row_c[:], in_=c_row[:])

        ones_c = small.tile([1, 1], bf16)
        nc.vector.memset(ones_c[:], 1.0)
        ones_row = small.tile([1, 128], bf16)
        nc.vector.memset(ones_row[:], 1.0)

        # x natural layout -> bf16, then broadcast on-chip to 128 partitions
        # via PE outer product (ones ⊗ x) + ScalarE PSUM->SBUF copies. Saves
        # 2 MiB of HBM traffic vs DMAing a pre-broadcast input.
        x_row = small.tile([1, S * N], f32)
        nc.sync.dma_start(x_row[:], x_d)
        x_row_c = small.tile([1, S * N], bf16)
        nc.scalar.copy(out=x_row_c[:], in_=x_row[:])
        xbc_all = small.tile([128, S * N], bf16)
        for s in range(S):
            for j in range(NCH):
                off = s * N + j * FREE
                xps = pps.tile([128, FREE], f32, tag=f"atl{j}", name=f"xps{s}_{j}")
                nc.tensor.matmul(
                    xps[:], lhsT=ones_row[:],
                    rhs=x_row_c[0:1, off:off + FREE],
                    start=True, stop=True,
                )
                nc.scalar.copy(out=xbc_all[:, off:off + FREE], in_=xps[:])

        ax_cols = small.tile([128, G], f32)
        stat_parts = small.tile([1, S * NCH], f32)
        act_junk = small.tile([1, FREE], f32)

        # part_cols layout: col 0 = dual (whole shard), cols 1..S = primal
        # per sample, cols S+1..2S = comp per sample. Host sums everything.
        part_cols = small.tile([128, 2 * S + 1], f32)

        # dual loss depends only on lambda -> compute up front, off the tail
        negr = small.tile([128, G], f32)
        nc.vector.tensor_scalar_min(negr[:], lam_cols[:], 0.0)
        junk_g = small.tile([128, G], f32)
        nc.vector.scalar_tensor_tensor(
            out=junk_g[:], in0=negr[:], scalar=1.0, in1=negr[:],
            op0=AOT.mult, op1=AOT.mult, accum_out=part_cols[:, 0:1])

        for s in range(S):
            xbc = xbc_all[:, s * N:(s + 1) * N]

            ps = [
                pps.tile([1, FREE], f32, tag=f"atl{j}", name=f"atl{j}")
                for j in range(NCH)
            ]
            for mc in range(MC):
                col = s * MC + mc
                atc = acast.tile([128, N], bf16, tag="atc")
                nc.gpsimd.dma_start(atc[:], a_d[s, mc * 128:(mc + 1) * 128, :])
                for j in range(NCH):
                    nc.tensor.matmul(
                        ps[j][:],
                        lhsT=lam_cols_c[:, col:col + 1],
                        rhs=atc[:, j * FREE:(j + 1) * FREE],
                        start=(mc == 0), stop=False,
                    )
                junk = scr.tile([128, N], bf16, tag="junk")
                nc.vector.scalar_tensor_tensor(
                    out=junk[:], in0=atc[:], scalar=1.0, in1=xbc[:],
                    op0=AOT.mult, op1=AOT.mult,
                    accum_out=ax_cols[:, col:col + 1],
                )
            for j in range(NCH):
                nc.tensor.matmul(
                    ps[j][:],
                    lhsT=ones_c[:],
                    rhs=c_row_c[0:1, s * N + j * FREE: s * N + (j + 1) * FREE],
                    start=False, stop=True,
                )
                nc.scalar.activation(
                    out=act_junk[:], in_=ps[j][:], func=AFT.Square,
                    accum_out=stat_parts[:, s * NCH + j: s * NCH + j + 1],
                )

            # per-sample finals on the [128, MC] slice -> overlap with the
            # next sample's stream instead of piling up at the kernel tail
            sl = slice(s * MC, (s + 1) * MC)
            axmb_s = small.tile([128, MC], f32, tag="axmb", name=f"axmb{s}")
            nc.vector.tensor_tensor(
                axmb_s[:], ax_cols[:, sl], b_cols[:, sl], AOT.subtract)
            relu_s = small.tile([128, MC], f32, tag="relu", name=f"relu{s}")
            nc.vector.tensor_scalar_max(relu_s[:], axmb_s[:], 0.0)
            junk_s = small.tile([128, MC], f32, tag="junks", name=f"junks{s}")
            nc.vector.scalar_tensor_tensor(
                out=junk_s[:], in0=relu_s[:], scalar=1.0, in1=relu_s[:],
                op0=AOT.mult, op1=AOT.mult,
                accum_out=part_cols[:, 1 + s:2 + s])
            lax_s = small.tile([128, MC], f32, tag="lax", name=f"lax{s}")
            nc.vector.tensor_tensor(
                lax_s[:], lam_cols[:, sl], axmb_s[:], AOT.mult)
            nc.vector.scalar_tensor_tensor(
                out=junk_s[:], in0=lax_s[:], scalar=1.0, in1=lax_s[:],
                op0=AOT.mult, op1=AOT.mult,
                accum_out=part_cols[:, 1 + S + s:2 + S + s])

        nc.sync.dma_start(part_d, part_cols[:])
        nc.sync.dma_start(stat_d.unsqueeze(0), stat_parts[:])

    nc.compile()
    return nc


def _prep_core_inputs(ci, A, x, lam, b, c):
    lo, hi = ci * S, (ci + 1) * S
    a_s = np.ascontiguousarray(A[lo:hi])
    xrow = np.ascontiguousarray(x[lo:hi].reshape(1, S * N))
    # [128, G] column layouts: cols[p, s*MC+mc] = v[s, mc*128 + p]
    lamcols = np.ascontiguousarray(lam[lo:hi].reshape(S * MC, 128).T)
    bcols = np.ascontiguousarray(b[lo:hi].reshape(S * MC, 128).T)
    crow = np.ascontiguousarray(c[lo:hi].reshape(1, S * N))
    return {
        "a_in": a_s, "x_in": xrow, "lamcols_in": lamcols,
        "bcols_in": bcols, "crow_in": crow,
    }


def kernel(x_hat, lam_hat, A, b_pad, c_pad, b_mask=None, c_mask=None):
    A = np.asarray(A, dtype=np.float32)
    x = np.asarray(x_hat, dtype=np.float32).reshape(B, N)
    lam = np.asarray(lam_hat, dtype=np.float32).reshape(B, M)
    b = np.asarray(b_pad, dtype=np.float32)
    c = np.asarray(c_pad, dtype=np.float32)

    if "nc" not in _CACHE:
        _CACHE["nc"] = _build()
    nc = _CACHE["nc"]

    in_maps = [_prep_core_inputs(ci, A, x, lam, b, c) for ci in range(N_CORES)]
    kw = {}
    if TRACE:
        kw["trace"] = True
    res = run_bass_kernel_spmd(nc, in_maps, core_ids=list(range(N_CORES)), **kw)
    kernel._last_result = res

    dual = primal = comp = stat = 0.0
    for r in res.results:
        part = r["part_out"].astype(np.float64)
        dual += part[:, 0].sum()
        primal += part[:, 1:1 + S].sum()
        comp += part[:, 1 + S:1 + 2 * S].sum()
        stat += r["stat_out"].astype(np.float64).sum()
    denom = float(B * M)
    total = (W_PRIMAL * primal + W_DUAL * dual
             + W_COMP * comp + W_STAT * stat) / denom
    return np.float32(total)


# revision 14
# speedup vs baseline: 1.0432x; 1.0432x over previous
"""KKT-loss Bass kernel for Trainium2, data-parallel over the batch on 8 cores.

Per core (4 samples of A [2048, 2048] fp32):
  - SWDGE DMA loads A in 1 MiB row-chunks [128, 2048], casting fp32 -> bf16
    inline (HBM reads stay fp32, so the kernel is honestly memory-bound on
    the fp32 tensor; SBUF receives bf16).
  - TensorE computes A^T @ lam with the bf16 chunk as the *moving* operand
    (contraction over the partition dim m), accumulating [1, 512] PSUM
    tiles across the 16 row-chunks; a final K=1 matmul adds c_pad into the
    accumulator, then ScalarE squares + reduces PSUM into per-(sample,
    n-chunk) scalars (the stat-loss partial sums).
  - VectorE computes A @ x via fused scalar_tensor_tensor (multiply by the
    row-broadcast x, reduce along the free dim) -> one [128, 1] column of
    Ax per chunk.
  - Small [128, 64]-shaped ops produce per-partition partial sums for the
    primal/dual/comp losses; the host combines all partials.

Host-side prep is layout-only sharding: batch split, x broadcast to 128
partitions (bf16), lambda/b reshaped to [128, 64] column layout.
"""

import os

import numpy as np
import ml_dtypes
from contextlib import ExitStack

import concourse.bacc as bacc
import concourse.tile as tile
import concourse.mybir as mybir
from concourse.bass_utils import run_bass_kernel_spmd

dt = mybir.dt
AOT = mybir.AluOpType
AFT = mybir.ActivationFunctionType

B, M, N = 32, 2048, 2048
N_CORES = 8
S = B // N_CORES            # 4 samples per core
MC = M // 128               # 16 row-chunks per sample
G = S * MC                  # 64 columns in the [128, G] layouts
FREE = 512
NCH = N // FREE             # 4 psum tiles per sample
W_PRIMAL, W_DUAL, W_STAT, W_COMP = 0.1, 0.1, 0.6, 0.2

TRACE = os.environ.get("KKT_TRACE", "0") == "1"

_CACHE = {}


def _build():
    f32 = dt.float32
    bf16 = dt.bfloat16

    nc = bacc.Bacc("TRN2", target_bir_lowering=False, debug=False)
    a_d = nc.dram_tensor("a_in", [S, M, N], f32, kind="ExternalInput").ap()
    xb_d = nc.dram_tensor("xb_in", [S, 128, N], bf16, kind="ExternalInput").ap()
    lamc_d = nc.dram_tensor("lamcols_in", [128, G], f32, kind="ExternalInput").ap()
    bc_d = nc.dram_tensor("bcols_in", [128, G], f32, kind="ExternalInput").ap()
    crow_d = nc.dram_tensor("crow_in", [1, S * N], f32, kind="ExternalInput").ap()
    part_d = nc.dram_tensor("part_out", [128, 2 * S + 1], f32,
                            kind="ExternalOutput").ap()
    stat_d = nc.dram_tensor("stat_out", [S * NCH], f32, kind="ExternalOutput").ap()

    with tile.TileContext(nc) as tc, ExitStack() as ctx:
        acast = ctx.enter_context(tc.tile_pool(name="ac", bufs=8))
        xpool = ctx.enter_context(tc.tile_pool(name="xp", bufs=2))
        scr = ctx.enter_context(tc.tile_pool(name="scr", bufs=2))
        small = ctx.enter_context(tc.tile_pool(name="small", bufs=1))
        pps = ctx.enter_context(tc.tile_pool(name="ps", bufs=2, space="PSUM"))

        lam_cols = small.tile([128, G], f32)
        nc.sync.dma_start(lam_cols[:], lamc_d)
        b_cols = small.tile([128, G], f32)
        nc.sync.dma_start(b_cols[:], bc_d)
        lam_cols_c = small.tile([128, G], bf16)
        nc.vector.tensor_copy(out=lam_cols_c[:], in_=lam_cols[:])

        c_row = small.tile([1, S * N], f32)
        nc.sync.dma_start(c_row[:], crow_d)
        c_row_c = small.tile([1, S * N], bf16)
        nc.scalar.copy(out=c_row_c[:], in_=c_row[:])

        ones_c = small.tile([1, 1], bf16)
        nc.vector.memset(ones_c[:], 1.0)

        ax_cols = small.tile([128, G], f32)
        stat_parts = small.tile([1, S * NCH], f32)
        act_junk = small.tile([1, FREE], f32)

        # part_cols layout: col 0 = dual (whole shard), cols 1..S = primal
        # per sample, cols S+1..2S = comp per sample. Host sums everything.
        part_cols = small.tile([128, 2 * S + 1], f32)

        # dual loss depends only on lambda -> compute up front, off the tail
        negr = small.tile([128, G], f32)
        nc.vector.tensor_scalar_min(negr[:], lam_cols[:], 0.0)
        junk_g = small.tile([128, G], f32)
        nc.vector.scalar_tensor_tensor(
            out=junk_g[:], in0=negr[:], scalar=1.0, in1=negr[:],
            op0=AOT.mult, op1=AOT.mult, accum_out=part_cols[:, 0:1])

        for s in range(S):
            xbc = xpool.tile([128, N], bf16, tag="xbc")
            nc.sync.dma_start(xbc[:], xb_d[s])

            ps = [
                pps.tile([1, FREE], f32, tag=f"atl{j}", name=f"atl{j}")
                for j in range(NCH)
            ]
            for mc in range(MC):
                col = s * MC + mc
                atc = acast.tile([128, N], bf16, tag="atc")
                nc.gpsimd.dma_start(atc[:], a_d[s, mc * 128:(mc + 1) * 128, :])
                for j in range(NCH):
                    nc.tensor.matmul(
                        ps[j][:],
                        lhsT=lam_cols_c[:, col:col + 1],
                        rhs=atc[:, j * FREE:(j + 1) * FREE],
                        start=(mc == 0), stop=False,
                    )
                junk = scr.tile([128, N], bf16, tag="junk")
                nc.vector.scalar_tensor_tensor(
                    out=junk[:], in0=atc[:], scalar=1.0, in1=xbc[:],
                    op0=AOT.mult, op1=AOT.mult,
                    accum_out=ax_cols[:, col:col + 1],
                )
            for j in range(NCH):
                nc.tensor.matmul(
                    ps[j][:],
                    lhsT=ones_c[:],
                    rhs=c_row_c[0:1, s * N + j * FREE: s * N + (j + 1) * FREE],
                    start=False, stop=True,
                )
                nc.scalar.activation(
                    out=act_junk[:], in_=ps[j][:], func=AFT.Square,
                    accum_out=stat_parts[:, s * NCH + j: s * NCH + j + 1],
                )

            # per-sample finals on the [128, MC] slice -> overlap with the
            # next sample's stream instead of piling up at the kernel tail
            sl = slice(s * MC, (s + 1) * MC)
            axmb_s = small.tile([128, MC], f32, tag="axmb", name=f"axmb{s}")
            nc.vector.tensor_tensor(
                axmb_s[:], ax_cols[:, sl], b_cols[:, sl], AOT.subtract)
            relu_s = small.tile([128, MC], f32, tag="relu", name=f"relu{s}")
            nc.vector.tensor_scalar_max(relu_s[:], axmb_s[:], 0.0)
            junk_s = small.tile([128, MC], f32, tag="junks", name=f"junks{s}")
            nc.vector.scalar_tensor_tensor(
                out=junk_s[:], in0=relu_s[:], scalar=1.0, in1=relu_s[:],
                op0=AOT.mult, op1=AOT.mult,
                accum_out=part_cols[:, 1 + s:2 + s])
            lax_s = small.tile([128, MC], f32, tag="lax", name=f"lax{s}")
            nc.vector.tensor_tensor(
                lax_s[:], lam_cols[:, sl], axmb_s[:], AOT.mult)
            nc.vector.scalar_tensor_tensor(
                out=junk_s[:], in0=lax_s[:], scalar=1.0, in1=lax_s[:],
                op0=AOT.mult, op1=AOT.mult,
                accum_out=part_cols[:, 1 + S + s:2 + S + s])

        nc.sync.dma_start(part_d, part_cols[:])
        nc.sync.dma_start(stat_d.unsqueeze(0), stat_parts[:])

    nc.compile()
    return nc


def _prep_core_inputs(ci, A, x, lam, b, c):
    lo, hi = ci * S, (ci + 1) * S
    a_s = np.ascontiguousarray(A[lo:hi])
    xb = np.ascontiguousarray(
        np.broadcast_to(
            x[lo:hi].astype(ml_dtypes.bfloat16)[:, None, :], (S, 128, N)))
    # [128, G] column layouts: cols[p, s*MC+mc] = v[s, mc*128 + p]
    lamcols = np.ascontiguousarray(lam[lo:hi].reshape(S * MC, 128).T)
    bcols = np.ascontiguousarray(b[lo:hi].reshape(S * MC, 128).T)
    crow = np.ascontiguousarray(c[lo:hi].reshape(1, S * N))
    return {
        "a_in": a_s, "xb_in": xb, "lamcols_in": lamcols,
        "bcols_in": bcols, "crow_in": crow,
    }


def kernel(x_hat, lam_hat, A, b_pad, c_pad, b_mask=None, c_mask=None):
    A = np.asarray(A, dtype=np.float32)
    x = np.asarray(x_hat, dtype=np.float32).reshape(B, N)
    lam = np.asarray(lam_hat, dtype=np.float32).reshape(B, M)
    b = np.asarray(b_pad, dtype=np.float32)
    c = np.asarray(c_pad, dtype=np.float32)

    if "nc" not in _CACHE:
        _CACHE["nc"] = _build()
    nc = _CACHE["nc"]

    in_maps = [_prep_core_inputs(ci, A, x, lam, b, c) for ci in range(N_CORES)]
    kw = {}
    if TRACE:
        kw["trace"] = True
    res = run_bass_kernel_spmd(nc, in_maps, core_ids=list(range(N_CORES)), **kw)
    kernel._last_result = res

    dual = primal = comp = stat = 0.0
    for r in res.results:
        part = r["part_out"].astype(np.float64)
        dual += part[:, 0].sum()
        primal += part[:, 1:1 + S].sum()
        comp += part[:, 1 + S:1 + 2 * S].sum()
        stat += r["stat_out"].astype(np.float64).sum()
    denom = float(B * M)
    total = (W_PRIMAL * primal + W_DUAL * dual
             + W_COMP * comp + W_STAT * stat) / denom
    return np.float32(total)
